# revision 1
# baseline (speedup 1.0000x reference)
"""Trainium2 Bass kernel for nn_CausalSelfAttention (B=2, T=2048, D=2048,
NH=16, NKV=4, HD=128, partial RoPE 64, per-head q_gain, ve_embed on V).

Sharding: 8 cores = (batch b in {0,1}) x (kv-head kv in {0..3}).
Core d = 4*b + kv computes q-heads [4kv..4kv+3] and kv-head kv for batch b:
  - QKV projections from pre-transposed x (fp16 matmuls, fp32 PSUM accum)
  - per-head RMS norm + partial RoPE + q_gain (fp32 vector math)
  - causal GQA attention computed transposed ([tk, tq] score blocks):
    softmax uses a global -32 shift instead of a row max (validated safe for
    randn-scaled inputs; exp stored bf16 whose f32-like exponent range
    absorbs the spread), so P needs no transpose before the PV matmul and
    the denominators come from a ones-matmul column reduction.
  - yT shard [512, T] -> AllGather within the 4-core batch group
  - column-parallel output projection: outT slice [512, T] per core
Host only shards/transpose-casts inputs and concatenates outputs.

The kernel is written to minimize instruction count and DMA count/bytes
(merged multi-head vector ops via strided/broadcast APs, one fat DMA per
tensor), which is what dominates both dispatch latency and HW time here.
"""

import math
import sys

import numpy as np

for _p in ("/opt/trn_rl_repo", "/root/.axon_site/_ro/trn_rl_repo"):
    if _p not in sys.path:
        sys.path.insert(0, _p)

import concourse.bass as bass
import concourse.mybir as mybir
import concourse.tile as tile
from concourse import bacc, bass_utils
from concourse.masks import make_identity

F16 = mybir.dt.float16
BF16 = mybir.dt.bfloat16
F32 = mybir.dt.float32
AX = mybir.AxisListType.X
AF = mybir.ActivationFunctionType

NH, NKV, HD = 16, 4, 128
B, T, D = 2, 2048, 2048
GH = NH // NKV          # 4 local q-heads per core
NS = GH + 1             # 5 norm/rope slots: 4 q-heads + k
TC = T // 128           # 16 t-chunks
DC = D // 128           # 16 d-chunks
QW = GH * HD            # 512 local q width
N_CORES = 8
RG = [[0, 1, 2, 3], [4, 5, 6, 7]]   # allgather groups = same batch
EPS = float(np.finfo(np.float32).eps)
CSHIFT = -32.0          # global softmax shift (replaces per-row max)

ts = bass.ts


def _emit_body(nc, tc, io):
    """One full forward pass for this core's shard."""
    xT, wqT, wkvT, wpT, ve, outT = (
        io["xT"], io["wqT"], io["wkvT"], io["wpT"], io["ve"], io["outT"],
    )
    ident, gsc_sb, msk_sb, cs_sb, sn_sb = (
        io["ident"], io["gsc_sb"], io["msk_sb"], io["cs_sb"], io["sn_sb"],
    )
    eps_sb, neg_sb, ones_sb, dram = (
        io["eps_sb"], io["neg_sb"], io["ones_sb"], io["dram"],
    )

    with tc.tile_pool(name="mid", bufs=1) as mid:
        # qkT_all: [hd, slot, t] fp16 — slots 0..3 = qT per head, slot 4 = kT
        qkT = mid.tile([128, NS, T], F16, name="qkT")
        vsb = mid.tile([128, TC, HD], BF16, name="vsb")
        yT = mid.tile([128, GH, T], F16, name="yT")
        ve_sb = mid.tile([128, TC, HD], F16, name="ve_sb")
        nc.sync.dma_start(
            ve_sb[:], ve.rearrange("(m p) f -> p m f", p=128)
        )

        # ---------------- phase 1: QKV projections + norm/rope ----------------
        with (
            tc.tile_pool(name="p1w", bufs=1) as p1w,
            tc.tile_pool(name="p1s", bufs=2) as scr,
            tc.tile_pool(name="p1q", bufs=2, space="PSUM") as psq,
            tc.tile_pool(name="p1tr", bufs=2, space="PSUM") as pstr,
        ):
            xsb = p1w.tile([128, DC, T], F16, name="xsb")
            wq_sb = p1w.tile([128, DC, QW], F16, name="wq_sb")
            wkv_sb = p1w.tile([128, DC, 2 * HD], F16, name="wkv_sb")
            nc.sync.dma_start(xsb[:], xT.rearrange("(c p) t -> p c t", p=128))
            nc.sync.dma_start(wq_sb[:], wqT.rearrange("(c p) i -> p c i", p=128))
            nc.sync.dma_start(wkv_sb[:], wkvT.rearrange("(c p) i -> p c i", p=128))

            for m in range(TC):
                pqkv = psq.tile([128, QW + 2 * HD], F32, name="pqkv")
                for c in range(DC):
                    st, sp = c == 0, c == DC - 1
                    xblk = xsb[:, c, ts(m, 128)]
                    nc.tensor.matmul(pqkv[:, 0:QW], xblk, wq_sb[:, c, :],
                                     start=st, stop=sp)
                    nc.tensor.matmul(pqkv[:, QW : QW + 2 * HD], xblk,
                                     wkv_sb[:, c, :], start=st, stop=sp)

                # rms factors for the 5 slots (4 q-heads + k) in one go
                qksq = scr.tile([128, NS * HD], F32, name="qksq")
                nc.scalar.square(qksq[:], pqkv[:, 0 : NS * HD])
                ssq = scr.tile([128, NS], F32, name="ssq")
                nc.vector.reduce_sum(
                    ssq[:], qksq[:].rearrange("p (s f) -> p s f", f=HD), axis=AX
                )
                srt = scr.tile([128, NS], F32, name="srt")
                nc.scalar.activation(srt[:], ssq[:], AF.Sqrt,
                                     bias=eps_sb[:, 0:1], scale=1.0 / HD)
                facs = scr.tile([128, NS], F32, name="facs")
                nc.vector.reciprocal(facs[:], srt[:])
                nc.vector.tensor_mul(facs[:], facs[:], gsc_sb[:])

                qkn = scr.tile([128, NS * HD], F32, name="qkn")
                nc.vector.tensor_mul(
                    qkn[:].rearrange("p (s f) -> p s f", f=HD),
                    pqkv[:, 0 : NS * HD].rearrange("p (s f) -> p s f", f=HD),
                    facs[:].to_broadcast((128, NS, HD)),
                )

                # v = proj + ve  (bf16, natural [t, hd] layout)
                nc.vector.tensor_add(vsb[:, m, :], pqkv[:, NS * HD : NS * HD + HD],
                                     ve_sb[:, m, :])

                # partial rope on dims 0:64 of each slot; all slots at once.
                # Operands in [p, freq, slot] order so cos/sin broadcast via a
                # trailing stride-0 dim.
                qkr = scr.tile([128, NS * HD], F16, name="qkr")
                qkn3 = qkn[:].rearrange("p (s f) -> p s f", f=HD)
                qkr3 = qkr[:].rearrange("p (s f) -> p s f", f=HD)
                xa = qkn3[:, :, 0:32].rearrange("p s f -> p f s")
                xb = qkn3[:, :, 32:64].rearrange("p s f -> p f s")
                cosb = cs_sb[:, ts(m, 32)].to_broadcast((128, 32, NS))
                sinb = sn_sb[:, ts(m, 32)].to_broadcast((128, 32, NS))
                t1 = scr.tile([128, 32, NS], F32, name="rt1")
                t2 = scr.tile([128, 32, NS], F32, name="rt2")
                nc.vector.tensor_mul(t1[:], xa, cosb)
                nc.vector.tensor_mul(t2[:], xb, sinb)
                nc.vector.tensor_sub(
                    qkr3[:, :, 0:32].rearrange("p s f -> p f s"), t1[:], t2[:]
                )
                nc.vector.tensor_mul(t1[:], xa, sinb)
                nc.vector.tensor_mul(t2[:], xb, cosb)
                nc.vector.tensor_add(
                    qkr3[:, :, 32:64].rearrange("p s f -> p f s"), t1[:], t2[:]
                )
                nc.vector.tensor_copy(qkr3[:, :, 64:HD], qkn3[:, :, 64:HD])

                # transpose the 5 slots into [hd, t] layout
                ptr = pstr.tile([128, NS, 128], F16, name="ptr")
                for s in range(NS):
                    nc.tensor.transpose(ptr[:, s, :], qkr[:, ts(s, 128)], ident[:])
                nc.vector.tensor_copy(qkT[:, :, ts(m, 128)], ptr[:])

        if io.get("stop_after") == "p1":
            nc.sync.dma_start(
                outT.rearrange("(c p) t -> p c t", p=128),
                qkT[:, 0:4, :],
            )
            return
        # ---------------- phase 2: causal GQA attention (transposed) ----------------
        with (
            tc.tile_pool(name="atp", bufs=1) as atp,
            tc.tile_pool(name="ats", bufs=2) as ats,
            tc.tile_pool(name="atps", bufs=1, space="PSUM") as pss,
            tc.tile_pool(name="atpy", bufs=2, space="PSUM") as psy,
            tc.tile_pool(name="atpd", bufs=2, space="PSUM") as psd,
        ):
            # pT[p, j, tq]: exp'd transposed scores, tk-chunk j on partitions.
            # Zeroed once; non-causal regions stay zero for all heads.
            pT = atp.tile([128, TC, T], BF16, name="pT")
            nc.vector.memset(pT[:], 0.0)
            for h in range(GH):
                for j in range(TC):
                    width = T - j * 128
                    psT = pss.tile([128, T], F32, name="psT")
                    for s in range((width + 511) // 512):
                        n = min(512, width - s * 512)
                        nc.tensor.matmul(
                            psT[:, s * 512 : s * 512 + n],
                            qkT[:, GH, ts(j, 128)],
                            qkT[:, h, j * 128 + s * 512 : j * 128 + s * 512 + n],
                            start=True, stop=True,
                        )
                    # mask the diagonal block (strictly-lower = future)
                    nc.vector.tensor_add(psT[:, 0:128], psT[:, 0:128], msk_sb[:])
                    nc.scalar.activation(pT[:, j, j * 128 : T], psT[:, 0:width],
                                         AF.Exp, bias=neg_sb[:, 0:1], scale=1.0)
                for g in range(4):
                    jn = 4 * g + 4
                    # denominators: sum over j (DVE) then over tk partitions
                    # (ones-matmul, broadcasting the result to all partitions)
                    jsum = ats.tile([128, 512], F32, name="jsum")
                    nc.vector.reduce_sum(
                        jsum[:],
                        pT[:, 0:jn, ts(g, 512)].rearrange("p j t -> p t j"),
                        axis=AX,
                    )
                    psums = psd.tile([128, 512], F32, name="psums")
                    nc.tensor.matmul(psums[:], ones_sb[:], jsum[:],
                                     start=True, stop=True)
                    rsb = ats.tile([128, 512], F32, name="rsb")
                    nc.vector.reciprocal(rsb[:], psums[:])
                    py = psy.tile([128, 512], F32, name="py")
                    for j in range(jn):
                        nc.tensor.matmul(py[:], vsb[:, j, :], pT[:, j, ts(g, 512)],
                                         start=(j == 0), stop=(j == jn - 1))
                    nc.vector.tensor_mul(yT[:, h, ts(g, 512)], py[:], rsb[:])

        if io.get("stop_after") == "attn":
            nc.sync.dma_start(
                outT.rearrange("(c p) t -> p c t", p=128), yT[:]
            )
            return
        # ---------------- phase 3: allgather y across the batch group ----------------
        bounce = dram.tile([QW, T], F16, name="bounce")
        nc.sync.dma_start(bounce.rearrange("(h p) t -> p h t", p=128), yT[:])
        gathered = dram.tile([NKV * QW, T], F16, name="gathered")
        if io.get("collective", True):
            nc.gpsimd.collective_compute(
                "AllGather",
                mybir.AluOpType.bypass,
                replica_groups=RG,
                ins=[bounce[:].opt()],
                outs=[gathered[:].opt()],
            )
        else:
            # timing/debug variant: fake the allgather with a local copy
            nc.sync.dma_start(gathered[0:QW, :], bounce[:])

    # ---------------- phase 4: column-parallel output projection ----------------
    with (
        tc.tile_pool(name="prw", bufs=1) as prw,
        tc.tile_pool(name="pro", bufs=2) as pro,
        tc.tile_pool(name="prp", bufs=2, space="PSUM") as pso,
    ):
        yf = prw.tile([128, DC, T], F16, name="yf")
        wp_sb = prw.tile([128, DC, QW], F16, name="wp_sb")
        nc.sync.dma_start(yf[:], gathered.rearrange("(c p) t -> p c t", p=128))
        nc.sync.dma_start(wp_sb[:], wpT.rearrange("(c p) i -> p c i", p=128))
        osb = pro.tile([128, 4, T], F16, name="osb")
        for c in range(4):
            po = pso.tile([128, T], F32, name="po")
            for j in range(DC):
                for t_ in range(4):
                    nc.tensor.matmul(
                        po[:, ts(t_, 512)],
                        wp_sb[:, j, ts(c, 128)],
                        yf[:, j, ts(t_, 512)],
                        start=(j == 0), stop=(j == DC - 1),
                    )
            nc.scalar.copy(osb[:, c, :], po[:])
        nc.sync.dma_start(outT.rearrange("(c p) t -> p c t", p=128), osb[:])


def _build(nreps=1, collective=True, compile=True, stop_after=None):
    nc = bacc.Bacc("TRN2", target_bir_lowering=False, debug=False,
                   num_devices=N_CORES)
    io = {
        "xT": nc.dram_tensor("xT", [D, T], F16, kind="ExternalInput").ap(),
        "wqT": nc.dram_tensor("wqT", [D, QW], F16, kind="ExternalInput").ap(),
        "wkvT": nc.dram_tensor("wkvT", [D, 2 * HD], F16, kind="ExternalInput").ap(),
        "wpT": nc.dram_tensor("wpT", [D, QW], F16, kind="ExternalInput").ap(),
        "ve": nc.dram_tensor("ve", [T, HD], F16, kind="ExternalInput").ap(),
        "cs": nc.dram_tensor("cs", [T, 32], F32, kind="ExternalInput").ap(),
        "sn": nc.dram_tensor("sn", [T, 32], F32, kind="ExternalInput").ap(),
        "gsc": nc.dram_tensor("gsc", [128, NS], F32, kind="ExternalInput").ap(),
        "msk": nc.dram_tensor("msk", [128, 128], F32, kind="ExternalInput").ap(),
        "outT": nc.dram_tensor("outT", [QW, T], F16, kind="ExternalOutput").ap(),
    }
    with tile.TileContext(nc) as tc:
        with (
            tc.tile_pool(name="persist", bufs=1) as pp,
            tc.tile_pool(name="dram", bufs=1, space="DRAM") as dram,
        ):
            ident = pp.tile([128, 128], F16, name="ident")
            make_identity(nc, ident)
            eps_sb = pp.tile([128, 1], F32, name="eps_sb")
            nc.vector.memset(eps_sb[:], EPS)
            neg_sb = pp.tile([128, 1], F32, name="neg_sb")
            nc.vector.memset(neg_sb[:], CSHIFT)
            ones_sb = pp.tile([128, 128], F32, name="ones_sb")
            nc.vector.memset(ones_sb[:], 1.0)
            gsc_sb = pp.tile([128, NS], F32, name="gsc_sb")
            nc.sync.dma_start(gsc_sb[:], io["gsc"][:])
            msk_sb = pp.tile([128, 128], F32, name="msk_sb")
            nc.sync.dma_start(msk_sb[:], io["msk"][:])
            # cos/sin as [128, TC*32]: chunk m in columns [m*32, (m+1)*32)
            cs_sb = pp.tile([128, TC * 32], F32, name="cs_sb")
            sn_sb = pp.tile([128, TC * 32], F32, name="sn_sb")
            nc.sync.dma_start(
                cs_sb[:].rearrange("p (m f) -> p m f", f=32),
                io["cs"].rearrange("(m p) f -> p m f", p=128),
            )
            nc.sync.dma_start(
                sn_sb[:].rearrange("p (m f) -> p m f", f=32),
                io["sn"].rearrange("(m p) f -> p m f", p=128),
            )
            io.update(ident=ident, gsc_sb=gsc_sb, msk_sb=msk_sb,
                      cs_sb=cs_sb, sn_sb=sn_sb, eps_sb=eps_sb, neg_sb=neg_sb,
                      ones_sb=ones_sb, dram=dram, collective=collective,
                      stop_after=stop_after)
            for _ in range(nreps):
                _emit_body(nc, tc, io)
    if compile:
        nc.compile()
    return nc


_NC_CACHE = {}


def _get_nc(nreps=1):
    if nreps not in _NC_CACHE:
        _NC_CACHE[nreps] = _build(nreps)
    return _NC_CACHE[nreps]


def _make_in_maps(x, ve_embed, Wq, Wk, Wv, Wproj, q_gain):
    f16, f32 = np.float16, np.float32
    inv_freq = 1.0 / (10000.0 ** (np.arange(0, HD, 2, dtype=f32) / HD))
    f = np.arange(T, dtype=f32)[:, None] * inv_freq[None, :]
    cs = np.ascontiguousarray(np.cos(f)[:, :32]).astype(f32)
    sn = np.ascontiguousarray(np.sin(f)[:, :32]).astype(f32)
    # transposed-scores diagonal-block mask: [tk, tq], future (tq < tk) = -1e30
    msk = np.where(
        np.arange(128)[None, :] >= np.arange(128)[:, None], 0.0, -1e30
    ).astype(f32)
    xTb = [np.ascontiguousarray(x[b].T).astype(f16) for b in range(B)]
    in_maps = []
    for d in range(N_CORES):
        b, kv = d // NKV, d % NKV
        gsc = np.ones(NS, f32)
        gsc[:GH] = q_gain[GH * kv : GH * (kv + 1)] / math.sqrt(HD)
        in_maps.append({
            "xT": xTb[b],
            "wqT": np.ascontiguousarray(
                Wq[GH * kv * HD : GH * (kv + 1) * HD, :].T).astype(f16),
            "wkvT": np.concatenate(
                [Wk[kv * HD : (kv + 1) * HD, :].T,
                 Wv[kv * HD : (kv + 1) * HD, :].T], axis=1).astype(f16),
            "wpT": np.ascontiguousarray(
                Wproj[kv * QW : (kv + 1) * QW, :].T).astype(f16),
            "ve": np.ascontiguousarray(
                ve_embed[b][:, kv * HD : (kv + 1) * HD]).astype(f16),
            "cs": cs,
            "sn": sn,
            "gsc": np.broadcast_to(gsc, (128, NS)).copy(),
            "msk": msk,
        })
    return in_maps


def _run(in_maps, nreps=1):
    nc = _get_nc(nreps)
    return bass_utils.run_bass_kernel_spmd(
        nc, in_maps, core_ids=list(range(N_CORES)), trace=False
    )


def kernel(x, ve_embed, Wq, Wk, Wv, Wproj, q_gain):
    x = np.asarray(x, np.float32)
    ve_embed = np.asarray(ve_embed, np.float32)
    Wq, Wk, Wv = (np.asarray(a, np.float32) for a in (Wq, Wk, Wv))
    Wproj = np.asarray(Wproj, np.float32)
    q_gain = np.asarray(q_gain, np.float32)

    in_maps = _make_in_maps(x, ve_embed, Wq, Wk, Wv, Wproj, q_gain)
    res = _run(in_maps, nreps=1)
    out = np.empty((B, T, D), np.float32)
    for d in range(N_CORES):
        b, kv = d // NKV, d % NKV
        out[b][:, kv * QW : (kv + 1) * QW] = res.results[d]["outT"].T.astype(
            np.float32)
    return out



# revision 3
# speedup vs baseline: 3.4707x; 3.4707x over previous
"""Trainium2 Bass kernel v2 for nn_CausalSelfAttention (B=2, T=2048, D=2048,
NH=16, NKV=4, HD=128, partial RoPE 64, per-head q_gain, ve_embed on V).

Sharding: 8 cores = (batch b in {0,1}) x (kv-head kv in {0..3}).

v2 vs v1: every vector/scalar op is a fat contiguous [<=128, 2048] op and
every DMA is contiguous per partition (host pre-lays all inputs):
  - q^T/k^T/v^T produced DIRECTLY by the projection matmuls (stationary =
    W^T chunk, moving = x^T chunk) -- no per-m-chunk transposes.
  - RMS norm factors via ones-matmul column sums; gain/eps/1/sqrt(HD) folded
    into per-slot scale/bias of a Sqrt activation + reciprocal.
  - RoPE on the transposed layout: partition-sliced fat ops with stacked
    cos/sin tables; two tiny SBUF->SBUF partition-shift DMAs per slot.
  - attention (transposed scores, global -32 softmax shift, ones-matmul
    denominators) unchanged from v1.
  - y^T allgathered within the batch group via flat p-major buffers; own
    slice copied from SBUF (no DMA); column-parallel output projection.
"""

import math
import sys

import numpy as np

for _p in ("/opt/trn_rl_repo", "/root/.axon_site/_ro/trn_rl_repo"):
    if _p not in sys.path:
        sys.path.insert(0, _p)

import concourse.bass as bass
import concourse.mybir as mybir
import concourse.tile as tile
from concourse import bacc, bass_utils
from concourse.masks import make_identity

F16 = mybir.dt.float16
BF16 = mybir.dt.bfloat16
F32 = mybir.dt.float32
AX = mybir.AxisListType.X
AF = mybir.ActivationFunctionType

NH, NKV, HD = 16, 4, 128
B, T, D = 2, 2048, 2048
GH = NH // NKV          # 4 local q-heads per core
NS = GH + 1             # 5 norm/rope slots: 4 q-heads + k
TC = T // 128           # 16 t-chunks
DC = D // 128           # 16 d-chunks
QW = GH * HD            # 512 local q width
N_CORES = 8
RG = [[0, 1, 2, 3], [4, 5, 6, 7]]   # allgather groups = same batch
EPS = float(np.finfo(np.float32).eps)
CSHIFT = -32.0          # global softmax shift (replaces per-row max)

ts = bass.ts


def _emit_body(nc, tc, io):
    """One full forward pass for this core's shard."""
    xc, wqk, wv, vet, wp, cssn, sncs, nsa, nsb, outp = (
        io["xc"], io["wqk"], io["wv"], io["vet"], io["wp"],
        io["cssn_sb"], io["sncs_sb"], io["nsa_sb"], io["nsb_sb"], io["outp"],
    )
    ident, msk_sb, neg_sb, ones_sb, dram = (
        io["ident"], io["msk_sb"], io["neg_sb"], io["ones_sb"], io["dram"],
    )

    with tc.tile_pool(name="mid", bufs=1) as mid:
        qkT = mid.tile([128, NS, T], F16, name="qkT")   # slots 0..3 qT, 4 kT
        vsb = mid.tile([128, TC, HD], BF16, name="vsb")  # v natural [t, hd]
        yT = mid.tile([128, GH, T], F16, name="yT")

        # ---------------- phase 1: QKV projections + norm/rope ----------------
        with (
            tc.tile_pool(name="p1w", bufs=1) as p1w,
            tc.tile_pool(name="p1s", bufs=1) as scr,
            tc.tile_pool(name="p1ps", bufs=1, space="PSUM") as psb,
            tc.tile_pool(name="p1pc", bufs=2, space="PSUM") as psc,
        ):
            xsb = p1w.tile([128, DC, T], F16, name="xsb")
            wqk_sb = p1w.tile([128, DC, NS * 128], F16, name="wqk_sb")
            wv_sb = p1w.tile([128, DC, HD], F16, name="wv_sb")
            vet_sb = p1w.tile([128, T], F16, name="vet_sb")
            nc.sync.dma_start(xsb[:], xc.rearrange("p (c t) -> p c t", t=T))
            nc.sync.dma_start(
                wqk_sb[:], wqk.rearrange("p (c i) -> p c i", i=NS * 128))
            nc.sync.dma_start(wv_sb[:], wv.rearrange("p (c i) -> p c i", i=HD))
            nc.sync.dma_start(vet_sb[:], vet[:])

            # --- 5 q/k slots: project (transposed), norm, rope ---
            for s in range(NS):
                psQ = psb.tile([128, T], F32, name="psQ")
                for c in range(DC):
                    st, sp = c == 0, c == DC - 1
                    for g in range(4):
                        nc.tensor.matmul(
                            psQ[:, ts(g, 512)], wqk_sb[:, c, ts(s, 128)],
                            xsb[:, c, ts(g, 512)], start=st, stop=sp,
                        )
                # column sums of squares -> norm factor (all rows equal)
                sq = scr.tile([128, T], F16, name="sq")
                nc.scalar.square(sq[:], psQ[:])
                fac = scr.tile([128, T], F32, name="fac")
                for g in range(4):
                    pc = psc.tile([128, 512], F32, name="pc")
                    nc.tensor.matmul(pc[:], ones_sb[:],
                                     sq[:, ts(g, 512)], start=True, stop=True)
                    nc.scalar.activation(fac[:, ts(g, 512)], pc[:], AF.Sqrt,
                                         bias=nsb[:, s : s + 1],
                                         scale=nsa[:, s : s + 1])
                nc.vector.reciprocal(fac[:], fac[:])

                # normalized rows 64:128 go straight to qkT
                nc.vector.tensor_mul(qkT[64:128, s, :], psQ[64:128, :],
                                     fac[64:128, :])
                # rows 0:64 -> rope: na = normalized [xa; xb]
                na = scr.tile([64, T], F16, name="na")
                nc.vector.tensor_mul(na[:], psQ[0:64, :], fac[0:64, :])
                u = scr.tile([64, T], F16, name="u")
                w = scr.tile([64, T], F16, name="w")
                nc.vector.tensor_mul(u[:], na[:], cssn[:])   # [xa*c; xb*s]
                nc.vector.tensor_mul(w[:], na[:], sncs[:])   # [xa*s; xb*c]
                # partition shifts so lanes align
                u2 = scr.tile([32, T], F16, name="u2")       # xb*s -> p0:32
                w2 = scr.tile([64, T], F16, name="w2")       # xa*s -> p32:64
                nc.sync.dma_start(u2[:], u[32:64, :])
                nc.sync.dma_start(w2[32:64, :], w[0:32, :])
                nc.vector.tensor_sub(qkT[0:32, s, :], u[0:32, :], u2[:])
                nc.vector.tensor_add(qkT[32:64, s, :], w2[32:64, :], w[32:64, :])

            # --- v: project transposed, add ve^T, transpose to natural ---
            psV = psb.tile([128, T], F32, name="psQ")
            for c in range(DC):
                st, sp = c == 0, c == DC - 1
                for g in range(4):
                    nc.tensor.matmul(psV[:, ts(g, 512)], wv_sb[:, c, :],
                                     xsb[:, c, ts(g, 512)], start=st, stop=sp)
            vT = scr.tile([128, T], F16, name="vT")
            nc.vector.tensor_add(vT[:], psV[:], vet_sb[:])
            for q4 in range(4):
                ptr = psc.tile([128, 4, 128], F16, name="ptr")
                for m4 in range(4):
                    nc.tensor.transpose(ptr[:, m4, :],
                                        vT[:, (q4 * 4 + m4) * 128 : (q4 * 4 + m4 + 1) * 128],
                                        ident[:])
                nc.vector.tensor_copy(
                    vsb[:, q4 * 4 : q4 * 4 + 4, :], ptr[:])

        if io.get("stop_after") == "p1":
            nc.sync.dma_start(outp.rearrange("p (c t) -> p c t", t=T),
                              qkT[:, 0:4, :])
            return
        # ---------------- phase 2: causal GQA attention (transposed) ----------------
        with (
            tc.tile_pool(name="atp", bufs=1) as atp,
            tc.tile_pool(name="ats", bufs=2) as ats,
            tc.tile_pool(name="atps", bufs=1, space="PSUM") as pss,
            tc.tile_pool(name="atpy", bufs=2, space="PSUM") as psy,
            tc.tile_pool(name="atpd", bufs=2, space="PSUM") as psd,
        ):
            # pT[p, j, tq]: exp'd transposed scores, tk-chunk j on partitions.
            pT = atp.tile([128, TC, T], BF16, name="pT")
            nc.vector.memset(pT[:], 0.0)
            for h in range(GH):
                for j in range(TC):
                    width = T - j * 128
                    psT = pss.tile([128, T], F32, name="psT")
                    for s in range((width + 511) // 512):
                        n = min(512, width - s * 512)
                        nc.tensor.matmul(
                            psT[:, s * 512 : s * 512 + n],
                            qkT[:, GH, ts(j, 128)],
                            qkT[:, h, j * 128 + s * 512 : j * 128 + s * 512 + n],
                            start=True, stop=True,
                        )
                    # mask the diagonal block (strictly-lower = future)
                    nc.vector.tensor_add(psT[:, 0:128], psT[:, 0:128], msk_sb[:])
                    nc.scalar.activation(pT[:, j, j * 128 : T], psT[:, 0:width],
                                         AF.Exp, bias=neg_sb[:, 0:1], scale=1.0)
                for g in range(4):
                    jn = 4 * g + 4
                    # denominators: DVE j-reduce then ones-matmul partition sum
                    jsum = ats.tile([128, 512], F32, name="jsum")
                    nc.vector.reduce_sum(
                        jsum[:],
                        pT[:, 0:jn, ts(g, 512)].rearrange("p j t -> p t j"),
                        axis=AX,
                    )
                    psums = psd.tile([128, 512], F32, name="psums")
                    nc.tensor.matmul(psums[:], io["ones32"][:], jsum[:],
                                     start=True, stop=True)
                    rsb = ats.tile([128, 512], F32, name="rsb")
                    nc.vector.reciprocal(rsb[:], psums[:])
                    py = psy.tile([128, 512], F32, name="py")
                    for j in range(jn):
                        nc.tensor.matmul(py[:], vsb[:, j, :], pT[:, j, ts(g, 512)],
                                         start=(j == 0), stop=(j == jn - 1))
                    nc.vector.tensor_mul(yT[:, h, ts(g, 512)], py[:], rsb[:])

        if io.get("stop_after") == "attn":
            nc.sync.dma_start(outp.rearrange("p (c t) -> p c t", t=T), yT[:])
            return
        # ---------------- phase 3: allgather y across the batch group ----------------
        bounce = dram.tile([128, GH * T], F16, name="bounce")
        nc.sync.dma_start(bounce.rearrange("p (h t) -> p h t", t=T), yT[:])
        gathered = dram.tile([NKV * 128, GH * T], F16, name="gathered")
        if io.get("collective", True):
            nc.gpsimd.collective_compute(
                "AllGather",
                mybir.AluOpType.bypass,
                replica_groups=RG,
                ins=[bounce[:].opt()],
                outs=[gathered[:].opt()],
            )
        else:
            nc.sync.dma_start(gathered[0:128, :], bounce[:])

        # ---------------- phase 4: column-parallel output projection ----------------
        with (
            tc.tile_pool(name="prw", bufs=1) as prw,
            tc.tile_pool(name="pro", bufs=2) as pro,
            tc.tile_pool(name="prp", bufs=2, space="PSUM") as pso,
        ):
            yf = prw.tile([128, NKV, GH * T], F16, name="yf")
            wp_sb = prw.tile([128, DC, 4 * 128], F16, name="wp_sb")
            nc.sync.dma_start(
                wp_sb[:], wp.rearrange("p (j i) -> p j i", i=4 * 128))
            # all 4 rank slices from gathered: per (p, r) 16KB contiguous
            nc.sync.dma_start(
                yf[:], gathered.rearrange("(r p) f -> p r f", p=128))
            yfv = yf[:].rearrange("p r (h t) -> p (r h) t", t=T)
            osb = pro.tile([128, 4, T], F16, name="osb")
            for co in range(4):
                po = pso.tile([128, T], F32, name="po")
                for j in range(DC):
                    for g in range(4):
                        nc.tensor.matmul(
                            po[:, ts(g, 512)],
                            wp_sb[:, j, ts(co, 128)],
                            yfv[:, j, ts(g, 512)],
                            start=(j == 0), stop=(j == DC - 1),
                        )
                nc.scalar.copy(osb[:, co, :], po[:])
            nc.sync.dma_start(outp.rearrange("p (c t) -> p c t", t=T), osb[:])


def _build(nreps=1, collective=True, compile=True, stop_after=None):
    nc = bacc.Bacc("TRN2", target_bir_lowering=False, debug=False,
                   num_devices=N_CORES)
    io = {
        "xc": nc.dram_tensor("xc", [128, DC * T], F16, kind="ExternalInput").ap(),
        "wqk": nc.dram_tensor("wqk", [128, DC * NS * 128], F16,
                              kind="ExternalInput").ap(),
        "wv": nc.dram_tensor("wv", [128, DC * HD], F16, kind="ExternalInput").ap(),
        "vet": nc.dram_tensor("vet", [128, T], F16, kind="ExternalInput").ap(),
        "wp": nc.dram_tensor("wp", [128, DC * 4 * 128], F16,
                             kind="ExternalInput").ap(),
        "cssn": nc.dram_tensor("cssn", [64, T], F32, kind="ExternalInput").ap(),
        "sncs": nc.dram_tensor("sncs", [64, T], F32, kind="ExternalInput").ap(),
        "nsa": nc.dram_tensor("nsa", [128, NS], F32, kind="ExternalInput").ap(),
        "nsb": nc.dram_tensor("nsb", [128, NS], F32, kind="ExternalInput").ap(),
        "msk": nc.dram_tensor("msk", [128, 128], F32, kind="ExternalInput").ap(),
        "outp": nc.dram_tensor("outp", [128, 4 * T], F16,
                               kind="ExternalOutput").ap(),
    }
    with tile.TileContext(nc) as tc:
        with (
            tc.tile_pool(name="persist", bufs=1) as pp,
            tc.tile_pool(name="dram", bufs=1, space="DRAM") as dram,
        ):
            ident = pp.tile([128, 128], F16, name="ident")
            make_identity(nc, ident)
            neg_sb = pp.tile([128, 1], F32, name="neg_sb")
            nc.vector.memset(neg_sb[:], CSHIFT)
            ones_sb = pp.tile([128, 128], F16, name="ones_sb")
            nc.vector.memset(ones_sb[:], 1.0)
            ones32 = pp.tile([128, 128], F32, name="ones32")
            nc.vector.memset(ones32[:], 1.0)
            msk_sb = pp.tile([128, 128], F32, name="msk_sb")
            nc.sync.dma_start(msk_sb[:], io["msk"][:])
            cssn_sb = pp.tile([64, T], F32, name="cssn_sb")
            sncs_sb = pp.tile([64, T], F32, name="sncs_sb")
            nc.sync.dma_start(cssn_sb[:], io["cssn"][:])
            nc.sync.dma_start(sncs_sb[:], io["sncs"][:])
            nsa_sb = pp.tile([128, NS], F32, name="nsa_sb")
            nsb_sb = pp.tile([128, NS], F32, name="nsb_sb")
            nc.sync.dma_start(nsa_sb[:], io["nsa"][:])
            nc.sync.dma_start(nsb_sb[:], io["nsb"][:])
            io.update(ident=ident, msk_sb=msk_sb, neg_sb=neg_sb,
                      ones_sb=ones_sb, cssn_sb=cssn_sb, sncs_sb=sncs_sb,
                      nsa_sb=nsa_sb, nsb_sb=nsb_sb, dram=dram, ones32=ones32,
                      collective=collective, stop_after=stop_after)
            for _ in range(nreps):
                _emit_body(nc, tc, io)
    if compile:
        nc.compile()
    return nc


_NC_CACHE = {}


def _get_nc(nreps=1):
    if nreps not in _NC_CACHE:
        _NC_CACHE[nreps] = _build(nreps)
    return _NC_CACHE[nreps]


def _dmajor(a):
    # [D, n] (d-major rows) -> [128, DC*n] with row p = chunks c of row c*128+p
    n = a.shape[1]
    return np.ascontiguousarray(
        a.reshape(DC, 128, n).transpose(1, 0, 2).reshape(128, DC * n))


def _make_in_maps(x, ve_embed, Wq, Wk, Wv, Wproj, q_gain):
    f16, f32 = np.float16, np.float32
    inv_freq = 1.0 / (10000.0 ** (np.arange(0, HD, 2, dtype=f32) / HD))
    f = np.arange(T, dtype=f32)[:, None] * inv_freq[None, :]  # [T, 64]
    cosT = np.ascontiguousarray(np.cos(f)[:, :32].T).astype(f32)  # [32, T]
    sinT = np.ascontiguousarray(np.sin(f)[:, :32].T).astype(f32)
    cssn = np.concatenate([cosT, sinT], axis=0)  # [64, T]
    sncs = np.concatenate([sinT, cosT], axis=0)
    msk = np.where(
        np.arange(128)[None, :] >= np.arange(128)[:, None], 0.0, -1e30
    ).astype(f32)
    xcb = [_dmajor(np.ascontiguousarray(x[b].T).astype(f16)) for b in range(B)]
    in_maps = []
    for d in range(N_CORES):
        b, kv = d // NKV, d % NKV
        # per-slot alpha: q slots gain/sqrt(HD), k slot 1
        alpha = np.ones(NS, f32)
        alpha[:GH] = q_gain[GH * kv : GH * (kv + 1)] / math.sqrt(HD)
        nsa = (1.0 / (HD * alpha ** 2)).astype(f32)
        nsb = (EPS / alpha ** 2).astype(f32)
        wqk_rows = np.concatenate(
            [Wq[GH * kv * HD : GH * (kv + 1) * HD, :],
             Wk[kv * HD : (kv + 1) * HD, :]], axis=0)  # [640, D]
        wp_slice = Wproj[kv * QW : (kv + 1) * QW, :]   # [512, D]
        wp = np.ascontiguousarray(
            wp_slice.reshape(4, 128, DC, 128).transpose(3, 2, 0, 1)
            .reshape(128, DC * 4 * 128)).astype(f16)
        in_maps.append({
            "xc": xcb[b],
            "wqk": _dmajor(np.ascontiguousarray(wqk_rows.T).astype(f16)),
            "wv": _dmajor(np.ascontiguousarray(
                Wv[kv * HD : (kv + 1) * HD, :].T).astype(f16)),
            "vet": np.ascontiguousarray(
                ve_embed[b][:, kv * HD : (kv + 1) * HD].T).astype(f16),
            "wp": wp,
            "cssn": cssn,
            "sncs": sncs,
            "nsa": np.broadcast_to(nsa, (128, NS)).copy(),
            "nsb": np.broadcast_to(nsb, (128, NS)).copy(),
            "msk": msk,
        })
    return in_maps


def _run(in_maps, nreps=1):
    nc = _get_nc(nreps)
    return bass_utils.run_bass_kernel_spmd(
        nc, in_maps, core_ids=list(range(N_CORES)), trace=False
    )


def kernel(x, ve_embed, Wq, Wk, Wv, Wproj, q_gain):
    x = np.asarray(x, np.float32)
    ve_embed = np.asarray(ve_embed, np.float32)
    Wq, Wk, Wv = (np.asarray(a, np.float32) for a in (Wq, Wk, Wv))
    Wproj = np.asarray(Wproj, np.float32)
    q_gain = np.asarray(q_gain, np.float32)

    in_maps = _make_in_maps(x, ve_embed, Wq, Wk, Wv, Wproj, q_gain)
    res = _run(in_maps, nreps=1)
    out = np.empty((B, T, D), np.float32)
    for d in range(N_CORES):
        b, kv = d // NKV, d % NKV
        o = res.results[d]["outp"].reshape(128, 4, T).astype(np.float32)
        out[b][:, kv * QW : (kv + 1) * QW] = o.transpose(2, 1, 0).reshape(T, QW)
    return out
